# revision 4
# baseline (speedup 1.0000x reference)
"""MoE (top-2 of 8 experts) Trainium2 kernel.

Strategy
--------
Data-parallel over tokens: 8 cores x 1792 tokens each.

Host prep (all linear / tiny):
  * gating MLP + softmax + top-2 (bit-identical jnp ops -> routing and the
    scalar load-balancing loss; ~2% of FLOPs, knife-edge top-k selection)
  * RevIN stats -> per-token affine (a, c); series_decomp moving average T0.
    Both commute with the expert linears:
      y[n] = sum_e g[n,e] * (a_n*((xs_n - T0_n) @ Ws[e].T + T0_n @ Wt[e].T)
                             + c_n * (Wt[e] @ 1))
    so tokens are pre-gathered per expert, pre-scaled by g*a, and the
    c-term is a tiny (N,8)@(8,512) matmul folded in on host.

Device (the ~95% of FLOPs):
  For every (token, chosen expert) pair, compute the 512x512 expert pair
  output via PE matmuls in float32r:
      O[j] = XSg[j] @ Ws[e].T + XTg[j] @ (Wt[e]-Ws[e]).T
  with activations stationary (128-token blocks) and weight l-stripes moving
  (N=512), accumulating 8 matmuls per block in one PSUM bank.

Combine: y[n] = O[r1(n)] + O[r2(n)] + cterm[n].
"""

import sys

sys.path.insert(0, "/opt/trn_rl_repo")

import numpy as np

BATCH = 2048
NV = 7
N = BATCH * NV
L = 512
P = 512
E = 8
K = 2
HID = 256
MA = 25
N_CORES = 8
SH = N // N_CORES  # tokens per core

_BASS_CACHE = {}


def _host_gating(xs, gw1, gw2, loss_coef):
    """Replicate the reference gating bit-for-bit (same jnp ops)."""
    import jax
    import jax.numpy as jnp

    xs = jnp.asarray(xs)
    clean_logits = jnp.maximum(xs @ jnp.asarray(gw1).T, 0.0) @ jnp.asarray(gw2).T
    probs = jax.nn.softmax(clean_logits, axis=1)
    top_vals, top_idx = jax.lax.top_k(probs, K + 1)
    tk_vals = top_vals[:, :K]
    tk_idx = top_idx[:, :K]
    tk_gates = tk_vals / (jnp.sum(tk_vals, axis=1, keepdims=True) + 1e-6)
    gates = jnp.zeros_like(probs).at[jnp.arange(N)[:, None], tk_idx].set(tk_gates)
    importance = jnp.sum(gates, axis=0)
    load = jnp.sum((gates > 0).astype(jnp.float32), axis=0)

    def _cv(v):
        return jnp.var(v, ddof=1) / (jnp.mean(v) ** 2 + 1e-10)

    loss = (_cv(importance) + _cv(load)) * loss_coef
    return (
        np.asarray(tk_idx),
        np.asarray(tk_gates),
        np.asarray(gates),
        np.asarray(loss),
    )


def _moving_avg(xs64):
    pad = (MA - 1) // 2
    xp = np.concatenate(
        [np.repeat(xs64[:, :1], pad, 1), xs64, np.repeat(xs64[:, -1:], pad, 1)], axis=1
    )
    cs = np.cumsum(np.pad(xp, ((0, 0), (1, 0))), axis=1)
    return (cs[:, MA:] - cs[:, :-MA]) / MA


def _build_bass(C, caps, reps=1):
    key = (C, tuple(caps), reps)
    if key in _BASS_CACHE:
        return _BASS_CACHE[key]

    import concourse.bass as bass  # noqa: F401
    import concourse.mybir as mybir
    import concourse.tile as tile
    from concourse import bacc

    F32 = mybir.dt.float32
    F32R = mybir.dt.float32r

    nc = bacc.Bacc(None, target_bir_lowering=False, debug=False)
    XSg = nc.dram_tensor("XSg", [L, C], F32R, kind="ExternalInput")
    XTg = nc.dram_tensor("XTg", [L, C], F32R, kind="ExternalInput")
    WST = nc.dram_tensor("WST", [E, L, P], F32R, kind="ExternalInput")
    WDT = nc.dram_tensor("WDT", [E, L, P], F32R, kind="ExternalInput")
    O = nc.dram_tensor("O", [C, P], F32, kind="ExternalOutput")

    NLT = L // 128  # l-stripes

    with tile.TileContext(nc) as tc:
        with (
            tc.tile_pool(name="acts", bufs=1) as acts,
            tc.tile_pool(name="wpool", bufs=2) as wpool,
            tc.tile_pool(name="opool", bufs=4) as opool,
            tc.tile_pool(name="psum", bufs=4, space="PSUM") as psum,
        ):
          for _rep in range(reps):
            xsg = acts.tile([128, NLT, C], F32R)
            xtg = acts.tile([128, NLT, C], F32R)
            for lt in range(NLT):
                nc.sync.dma_start(out=xsg[:, lt, :], in_=XSg[lt * 128:(lt + 1) * 128, :])
                nc.sync.dma_start(out=xtg[:, lt, :], in_=XTg[lt * 128:(lt + 1) * 128, :])

            col0 = 0
            for e in range(E):
                ws = wpool.tile([128, NLT, P], F32R, tag="ws")
                wd = wpool.tile([128, NLT, P], F32R, tag="wd")
                for lt in range(NLT):
                    nc.sync.dma_start(out=ws[:, lt, :], in_=WST[e, lt * 128:(lt + 1) * 128, :])
                    nc.sync.dma_start(out=wd[:, lt, :], in_=WDT[e, lt * 128:(lt + 1) * 128, :])
                for b in range(caps[e] // 128):
                    c0 = col0 + b * 128
                    acc = psum.tile([128, P], F32)
                    for lt in range(NLT):
                        nc.tensor.matmul(
                            acc[:], xsg[:, lt, c0:c0 + 128], ws[:, lt, :],
                            start=(lt == 0), stop=False,
                        )
                    for lt in range(NLT):
                        nc.tensor.matmul(
                            acc[:], xtg[:, lt, c0:c0 + 128], wd[:, lt, :],
                            start=False, stop=(lt == NLT - 1),
                        )
                    ot = opool.tile([128, P], F32)
                    nc.any.tensor_copy(ot[:], acc[:])
                    nc.sync.dma_start(out=O[c0:c0 + 128, :], in_=ot[:])
                col0 += caps[e]

    nc.compile()
    _BASS_CACHE[key] = nc
    return nc


def kernel(x, gw1, gw2, Ws, Wt, revin_w, revin_b, loss_coef):
    from concourse.bass_utils import run_bass_kernel_spmd

    x = np.asarray(x)
    gw1 = np.asarray(gw1)
    gw2 = np.asarray(gw2)
    Ws = np.asarray(Ws)
    Wt = np.asarray(Wt)
    revin_w = np.asarray(revin_w)
    revin_b = np.asarray(revin_b)

    xs = x[..., 0]  # (N, L) f32

    # ---- host: gating + loss (bit-identical to reference) ----
    tk_idx, tk_gates, gates, loss = _host_gating(xs, gw1, gw2, loss_coef)

    # ---- host: RevIN affine + decomposition (linear prep, f64) ----
    xs64 = xs.astype(np.float64)
    mu = xs64.mean(axis=1)
    var = xs64.var(axis=1)
    sd = np.sqrt(var + 1e-5)
    y_idx = np.arange(N) % NV
    a_tok = (revin_w.astype(np.float64)[y_idx] / sd)
    c_tok = (revin_b.astype(np.float64)[y_idx] - mu * a_tok)
    T0 = _moving_avg(xs64)
    S0 = xs64 - T0

    # ---- host: routing arrays per core ----
    counts = np.zeros((N_CORES, E), np.int64)
    for c in range(N_CORES):
        sl = slice(c * SH, (c + 1) * SH)
        counts[c] = (
            np.bincount(tk_idx[sl, 0], minlength=E)
            + np.bincount(tk_idx[sl, 1], minlength=E)
        )
    caps = ((counts.max(axis=0) + 127) // 128 * 128).astype(np.int64)
    caps = np.maximum(caps, 128)
    C = int(caps.sum())
    seg0 = np.concatenate([[0], np.cumsum(caps)])[:E]

    XSg_all = np.empty((N_CORES, L, C), np.float32)
    XTg_all = np.empty((N_CORES, L, C), np.float32)
    r1_all = np.empty((N_CORES, SH), np.int64)
    r2_all = np.empty((N_CORES, SH), np.int64)
    for c in range(N_CORES):
        sl = slice(c * SH, (c + 1) * SH)
        ti = tk_idx[sl]
        tg = tk_gates[sl]
        perm = np.zeros(C, np.int64)
        gsc = np.zeros(C, np.float64)
        for e in range(E):
            l1 = np.nonzero(ti[:, 0] == e)[0]
            l2 = np.nonzero(ti[:, 1] == e)[0]
            seg = np.concatenate([l1, l2])
            o = seg0[e]
            perm[o:o + len(seg)] = seg
            gsc[o:o + len(l1)] = tg[l1, 0]
            gsc[o + len(l1):o + len(seg)] = tg[l2, 1]
            r1_all[c, l1] = o + np.arange(len(l1))
            r2_all[c, l2] = o + len(l1) + np.arange(len(l2))
        ga = gsc * a_tok[sl][perm]
        ga[np.concatenate([np.arange(seg0[e] + counts[c, e], seg0[e] + caps[e])
                           for e in range(E)])] = 0.0
        XSg_all[c] = (S0[sl][perm] * ga[:, None]).T.astype(np.float32)
        XTg_all[c] = (T0[sl][perm] * ga[:, None]).T.astype(np.float32)

    U = Wt.sum(axis=2).astype(np.float64)  # (E, P)
    Gc = gates.astype(np.float64) * c_tok[:, None]  # (N, E)
    cterm = (Gc @ U)  # (N, P) f64, tiny matmul

    WST = np.ascontiguousarray(Ws.transpose(0, 2, 1)).astype(np.float32)
    WDT = np.ascontiguousarray(Wt.transpose(0, 2, 1)).astype(np.float32)

    # ---- device: expert pair outputs ----
    nc = _build_bass(C, tuple(int(v) for v in caps))
    in_maps = [
        {"XSg": XSg_all[c], "XTg": XTg_all[c], "WST": WST, "WDT": WDT}
        for c in range(N_CORES)
    ]
    res = run_bass_kernel_spmd(nc, in_maps, list(range(N_CORES)))

    # ---- host: combine ----
    y = np.empty((N, P), np.float32)
    for c in range(N_CORES):
        O = res.results[c]["O"]  # (C, P) f32
        sl = slice(c * SH, (c + 1) * SH)
        y[sl] = (
            O[r1_all[c]].astype(np.float64)
            + O[r2_all[c]].astype(np.float64)
            + cterm[sl]
        ).astype(np.float32)

    return y[..., None].astype(np.float32), loss


# revision 6
# speedup vs baseline: 27583.1576x; 27583.1576x over previous
"""MoE (top-2 of 8 experts) Trainium2 kernel.

Strategy
--------
Data-parallel over tokens: 8 cores x 1792 tokens each.

Host prep (all linear / tiny):
  * gating MLP + softmax + top-2 (bit-identical jnp ops -> routing and the
    scalar load-balancing loss; ~2% of FLOPs, knife-edge top-k selection)
  * RevIN stats -> per-token affine (a, c); series_decomp moving average T0.
    Both commute with the expert linears:
      y[n] = sum_e g[n,e] * (a_n*((xs_n - T0_n) @ Ws[e].T + T0_n @ Wt[e].T)
                             + c_n * (Wt[e] @ 1))
    so tokens are pre-gathered per expert, pre-scaled by g*a, and the
    c-term is a tiny (N,8)@(8,512) matmul folded in on host.

Device (the ~95% of FLOPs):
  For every (token, chosen expert) pair, compute the 512x512 expert pair
  output via PE matmuls in float16 (fp32 PSUM accumulation):
      O[j] = XSg[j] @ Ws[e].T + XTg[j] @ Wt[e].T
  with activations stationary (128-token blocks) and weight l-stripes moving
  (N=512), accumulating 8 matmuls per block in one PSUM bank.

Combine: y[n] = O[r1(n)] + O[r2(n)] + cterm[n].
"""

import sys

sys.path.insert(0, "/opt/trn_rl_repo")

import numpy as np

BATCH = 2048
NV = 7
N = BATCH * NV
L = 512
P = 512
E = 8
K = 2
HID = 256
MA = 25
N_CORES = 8
SH = N // N_CORES  # tokens per core

_BASS_CACHE = {}


def _host_gating(xs, gw1, gw2, loss_coef):
    """Replicate the reference gating bit-for-bit (same jnp ops)."""
    import jax
    import jax.numpy as jnp

    xs = jnp.asarray(xs)
    clean_logits = jnp.maximum(xs @ jnp.asarray(gw1).T, 0.0) @ jnp.asarray(gw2).T
    probs = jax.nn.softmax(clean_logits, axis=1)
    top_vals, top_idx = jax.lax.top_k(probs, K + 1)
    tk_vals = top_vals[:, :K]
    tk_idx = top_idx[:, :K]
    tk_gates = tk_vals / (jnp.sum(tk_vals, axis=1, keepdims=True) + 1e-6)
    gates = jnp.zeros_like(probs).at[jnp.arange(N)[:, None], tk_idx].set(tk_gates)
    importance = jnp.sum(gates, axis=0)
    load = jnp.sum((gates > 0).astype(jnp.float32), axis=0)

    def _cv(v):
        return jnp.var(v, ddof=1) / (jnp.mean(v) ** 2 + 1e-10)

    loss = (_cv(importance) + _cv(load)) * loss_coef
    return (
        np.asarray(tk_idx),
        np.asarray(tk_gates),
        np.asarray(gates),
        np.asarray(loss),
    )


def _moving_avg(xs64):
    pad = (MA - 1) // 2
    xp = np.concatenate(
        [np.repeat(xs64[:, :1], pad, 1), xs64, np.repeat(xs64[:, -1:], pad, 1)], axis=1
    )
    cs = np.cumsum(np.pad(xp, ((0, 0), (1, 0))), axis=1)
    return (cs[:, MA:] - cs[:, :-MA]) / MA


def _build_bass(C, caps, reps=1):
    key = (C, tuple(caps), reps)
    if key in _BASS_CACHE:
        return _BASS_CACHE[key]

    import concourse.mybir as mybir
    import concourse.tile as tile
    from concourse import bacc

    F32 = mybir.dt.float32
    F16 = mybir.dt.float16
    act_dt = w_dt = out_dt = F16
    NLT = L // 128
    seg0 = np.concatenate([[0], np.cumsum(caps)])[:E]
    capmax = int(max(caps))
    evict_batch = 2

    nc = bacc.Bacc(None, target_bir_lowering=False, debug=False)
    XSg = nc.dram_tensor("XSg", [NLT, 128, C], act_dt, kind="ExternalInput")
    XTg = nc.dram_tensor("XTg", [NLT, 128, C], act_dt, kind="ExternalInput")
    WST = nc.dram_tensor("WST", [E, NLT, 128, P], w_dt, kind="ExternalInput")
    WDT = nc.dram_tensor("WDT", [E, NLT, 128, P], w_dt, kind="ExternalInput")
    O = nc.dram_tensor("O", [C, P], out_dt, kind="ExternalOutput")

    with tile.TileContext(nc) as tc:
        with (
            tc.tile_pool(name="acts", bufs=3) as acts,
            tc.tile_pool(name="wpool", bufs=2) as wpool,
            tc.tile_pool(name="opool", bufs=3) as opool,
            tc.tile_pool(name="psum", bufs=8, space="PSUM") as psum,
        ):
          for _rep in range(reps):
            for e in range(E):
                cap = int(caps[e])
                col0 = int(seg0[e])
                ws = wpool.tile([128, NLT, P], w_dt, tag="ws")
                wd = wpool.tile([128, NLT, P], w_dt, tag="wd")
                nc.sync.dma_start(out=ws[:], in_=WST[e].rearrange("a p c -> p a c"))
                nc.sync.dma_start(out=wd[:], in_=WDT[e].rearrange("a p c -> p a c"))
                xsg = acts.tile([128, NLT, capmax], act_dt, tag="xsg")
                xtg = acts.tile([128, NLT, capmax], act_dt, tag="xtg")
                nc.sync.dma_start(
                    out=xsg[:, :, :cap],
                    in_=XSg[:, :, col0:col0 + cap].rearrange("a p c -> p a c"),
                )
                nc.sync.dma_start(
                    out=xtg[:, :, :cap],
                    in_=XTg[:, :, col0:col0 + cap].rearrange("a p c -> p a c"),
                )
                nblk = cap // 128
                for bg in range(0, nblk, evict_batch):
                    bcnt = min(evict_batch, nblk - bg)
                    ot = opool.tile([128, evict_batch, P], out_dt, tag="ot")
                    for bi in range(bcnt):
                        c0 = (bg + bi) * 128
                        acc = psum.tile([128, P], F32)
                        for lt in range(NLT):
                            nc.tensor.matmul(acc[:], xsg[:, lt, c0:c0 + 128], ws[:, lt, :],
                                             start=(lt == 0), stop=False)
                        for lt in range(NLT):
                            nc.tensor.matmul(acc[:], xtg[:, lt, c0:c0 + 128], wd[:, lt, :],
                                             start=False, stop=(lt == NLT - 1))
                        nc.any.tensor_copy(ot[:, bi, :], acc[:])
                    oc0 = col0 + bg * 128
                    nc.gpsimd.dma_start(
                        out=O[oc0:oc0 + bcnt * 128, :].rearrange("(b p) c -> p b c", p=128),
                        in_=ot[:, :bcnt, :],
                    )

    nc.compile()
    _BASS_CACHE[key] = nc
    return nc


def kernel(x, gw1, gw2, Ws, Wt, revin_w, revin_b, loss_coef):
    from concourse.bass_utils import run_bass_kernel_spmd

    x = np.asarray(x)
    gw1 = np.asarray(gw1)
    gw2 = np.asarray(gw2)
    Ws = np.asarray(Ws)
    Wt = np.asarray(Wt)
    revin_w = np.asarray(revin_w)
    revin_b = np.asarray(revin_b)

    xs = x[..., 0]  # (N, L) f32

    # ---- host: gating + loss (bit-identical to reference) ----
    tk_idx, tk_gates, gates, loss = _host_gating(xs, gw1, gw2, loss_coef)

    # ---- host: RevIN affine + decomposition (linear prep, f64) ----
    xs64 = xs.astype(np.float64)
    mu = xs64.mean(axis=1)
    var = xs64.var(axis=1)
    sd = np.sqrt(var + 1e-5)
    y_idx = np.arange(N) % NV
    a_tok = (revin_w.astype(np.float64)[y_idx] / sd)
    c_tok = (revin_b.astype(np.float64)[y_idx] - mu * a_tok)
    T0 = _moving_avg(xs64)
    S0 = xs64 - T0

    # ---- host: routing arrays per core ----
    counts = np.zeros((N_CORES, E), np.int64)
    for c in range(N_CORES):
        sl = slice(c * SH, (c + 1) * SH)
        counts[c] = (
            np.bincount(tk_idx[sl, 0], minlength=E)
            + np.bincount(tk_idx[sl, 1], minlength=E)
        )
    caps = ((counts.max(axis=0) + 127) // 128 * 128).astype(np.int64)
    caps = np.maximum(caps, 128)
    C = int(caps.sum())
    seg0 = np.concatenate([[0], np.cumsum(caps)])[:E]

    XSg_all = np.empty((N_CORES, L, C), np.float16)
    XTg_all = np.empty((N_CORES, L, C), np.float16)
    r1_all = np.empty((N_CORES, SH), np.int64)
    r2_all = np.empty((N_CORES, SH), np.int64)
    for c in range(N_CORES):
        sl = slice(c * SH, (c + 1) * SH)
        ti = tk_idx[sl]
        tg = tk_gates[sl]
        perm = np.zeros(C, np.int64)
        gsc = np.zeros(C, np.float64)
        for e in range(E):
            l1 = np.nonzero(ti[:, 0] == e)[0]
            l2 = np.nonzero(ti[:, 1] == e)[0]
            seg = np.concatenate([l1, l2])
            o = seg0[e]
            perm[o:o + len(seg)] = seg
            gsc[o:o + len(l1)] = tg[l1, 0]
            gsc[o + len(l1):o + len(seg)] = tg[l2, 1]
            r1_all[c, l1] = o + np.arange(len(l1))
            r2_all[c, l2] = o + len(l1) + np.arange(len(l2))
        ga = gsc * a_tok[sl][perm]
        ga[np.concatenate([np.arange(seg0[e] + counts[c, e], seg0[e] + caps[e])
                           for e in range(E)])] = 0.0
        XSg_all[c] = (S0[sl][perm] * ga[:, None]).T.astype(np.float16)
        XTg_all[c] = (T0[sl][perm] * ga[:, None]).T.astype(np.float16)

    U = Wt.sum(axis=2).astype(np.float64)  # (E, P)
    Gc = gates.astype(np.float64) * c_tok[:, None]  # (N, E)
    cterm = (Gc @ U)  # (N, P) f64, tiny matmul

    WST = np.ascontiguousarray(Ws.transpose(0, 2, 1)).astype(np.float16).reshape(E, L // 128, 128, P)
    WDT = np.ascontiguousarray(Wt.transpose(0, 2, 1)).astype(np.float16).reshape(E, L // 128, 128, P)

    # ---- device: expert pair outputs ----
    nc = _build_bass(C, tuple(int(v) for v in caps))
    in_maps = [
        {
            "XSg": XSg_all[c].reshape(L // 128, 128, C),
            "XTg": XTg_all[c].reshape(L // 128, 128, C),
            "WST": WST,
            "WDT": WDT,
        }
        for c in range(N_CORES)
    ]
    res = run_bass_kernel_spmd(nc, in_maps, list(range(N_CORES)))

    # ---- host: combine ----
    y = np.empty((N, P), np.float32)
    for c in range(N_CORES):
        O = res.results[c]["O"]  # (C, P) f32
        sl = slice(c * SH, (c + 1) * SH)
        y[sl] = (
            O[r1_all[c]].astype(np.float64)
            + O[r2_all[c]].astype(np.float64)
            + cterm[sl]
        ).astype(np.float32)

    return y[..., None].astype(np.float32), loss


# revision 7
# speedup vs baseline: 28648.2521x; 1.0386x over previous
"""MoE Trainium2 kernel v3: expert-balanced pair sharding, per-core programs.

All (token, expert) pairs are grouped by expert into 128-row blocks globally,
then the block list is cut into 8 equal contiguous spans (one per core).
Each core runs its OWN compiled program (runs of consecutive same-expert
blocks -> one weight load per run), dispatched concurrently on its pinned
device via threads. Host combines y[n] = O[r1]+O[r2]+cterm as before.
"""

import sys

sys.path.insert(0, "/opt/trn_rl_repo")

import threading

import numpy as np

BATCH = 2048
NV = 7
N = BATCH * NV
L = 512
P = 512
E = 8
K = 2
HID = 256
MA = 25
N_CORES = 8

_BASS_CACHE = {}


def _host_gating(xs, gw1, gw2, loss_coef):
    """Replicate the reference gating bit-for-bit (same jnp ops)."""
    import jax
    import jax.numpy as jnp

    xs = jnp.asarray(xs)
    clean_logits = jnp.maximum(xs @ jnp.asarray(gw1).T, 0.0) @ jnp.asarray(gw2).T
    probs = jax.nn.softmax(clean_logits, axis=1)
    top_vals, top_idx = jax.lax.top_k(probs, K + 1)
    tk_vals = top_vals[:, :K]
    tk_idx = top_idx[:, :K]
    tk_gates = tk_vals / (jnp.sum(tk_vals, axis=1, keepdims=True) + 1e-6)
    gates = jnp.zeros_like(probs).at[jnp.arange(N)[:, None], tk_idx].set(tk_gates)
    importance = jnp.sum(gates, axis=0)
    load = jnp.sum((gates > 0).astype(jnp.float32), axis=0)

    def _cv(v):
        return jnp.var(v, ddof=1) / (jnp.mean(v) ** 2 + 1e-10)

    loss = (_cv(importance) + _cv(load)) * loss_coef
    return (
        np.asarray(tk_idx),
        np.asarray(tk_gates),
        np.asarray(gates),
        np.asarray(loss),
    )


def _moving_avg(xs64):
    pad = (MA - 1) // 2
    xp = np.concatenate(
        [np.repeat(xs64[:, :1], pad, 1), xs64, np.repeat(xs64[:, -1:], pad, 1)], axis=1
    )
    cs = np.cumsum(np.pad(xp, ((0, 0), (1, 0))), axis=1)
    return (cs[:, MA:] - cs[:, :-MA]) / MA


def _build_core_program(runs, C_core, chunk_blocks=2, evict_batch=2, core=0):
    """One core's program. runs = tuple of (nblk,) per weight-run."""
    key = (tuple(runs), C_core, chunk_blocks, evict_batch, core)
    if key in _BASS_CACHE:
        return _BASS_CACHE[key]

    import concourse.mybir as mybir
    import concourse.tile as tile
    from concourse import bacc

    F32 = mybir.dt.float32
    F16 = mybir.dt.float16
    NLT = L // 128
    n_runs = len(runs)

    nc = bacc.Bacc(None, target_bir_lowering=False, debug=False)
    XS = nc.dram_tensor("XS", [NLT, 128, C_core], F16, kind="ExternalInput")
    XT = nc.dram_tensor("XT", [NLT, 128, C_core], F16, kind="ExternalInput")
    W = nc.dram_tensor("W", [n_runs, 2, NLT, 128, P], F16, kind="ExternalInput")
    O = nc.dram_tensor("O", [C_core, P], F16, kind="ExternalOutput")

    with tile.TileContext(nc) as tc:
        with (
            tc.tile_pool(name="acts", bufs=3) as acts,
            tc.tile_pool(name="wpool", bufs=2) as wpool,
            tc.tile_pool(name="opool", bufs=3) as opool,
            tc.tile_pool(name="psum", bufs=8, space="PSUM") as psum,
        ):
            blk0 = 0
            for r, nblk in enumerate(runs):
                w = wpool.tile([128, 2, NLT, P], F16, tag="w")
                nc.sync.dma_start(out=w[:], in_=W[r].rearrange("s a p c -> p s a c"))
                for cb in range(0, nblk, chunk_blocks):
                    ncb = min(chunk_blocks, nblk - cb)
                    cols = ncb * 128
                    col0 = (blk0 + cb) * 128
                    xs_t = acts.tile([128, NLT, chunk_blocks * 128], F16, tag="xs")
                    xt_t = acts.tile([128, NLT, chunk_blocks * 128], F16, tag="xt")
                    nc.sync.dma_start(
                        out=xs_t[:, :, :cols],
                        in_=XS[:, :, col0:col0 + cols].rearrange("a p c -> p a c"),
                    )
                    nc.sync.dma_start(
                        out=xt_t[:, :, :cols],
                        in_=XT[:, :, col0:col0 + cols].rearrange("a p c -> p a c"),
                    )
                    for bg in range(0, ncb, evict_batch):
                        bcnt = min(evict_batch, ncb - bg)
                        ot = opool.tile([128, evict_batch, P], F16, tag="ot")
                        for bi in range(bcnt):
                            c0 = (bg + bi) * 128
                            acc = psum.tile([128, P], F32)
                            for lt in range(NLT):
                                nc.tensor.matmul(
                                    acc[:], xs_t[:, lt, c0:c0 + 128], w[:, 0, lt, :],
                                    start=(lt == 0), stop=False,
                                )
                            for lt in range(NLT):
                                nc.tensor.matmul(
                                    acc[:], xt_t[:, lt, c0:c0 + 128], w[:, 1, lt, :],
                                    start=False, stop=(lt == NLT - 1),
                                )
                            nc.any.tensor_copy(ot[:, bi, :], acc[:])
                        oc0 = col0 + bg * 128
                        nc.gpsimd.dma_start(
                            out=O[oc0:oc0 + bcnt * 128, :].rearrange(
                                "(b p) c -> p b c", p=128
                            ),
                            in_=ot[:, :bcnt, :],
                        )
                blk0 += nblk

    nc.compile()
    _BASS_CACHE[key] = nc
    return nc


def kernel(x, gw1, gw2, Ws, Wt, revin_w, revin_b, loss_coef):
    import jax
    from concourse import bass2jax

    x = np.asarray(x)
    gw1 = np.asarray(gw1)
    gw2 = np.asarray(gw2)
    Ws = np.asarray(Ws)
    Wt = np.asarray(Wt)
    revin_w = np.asarray(revin_w)
    revin_b = np.asarray(revin_b)

    xs = x[..., 0]  # (N, L) f32

    # ---- host: gating + loss (bit-identical to reference) ----
    tk_idx, tk_gates, gates, loss = _host_gating(xs, gw1, gw2, loss_coef)

    # ---- host: RevIN affine + decomposition (linear prep, f64) ----
    xs64 = xs.astype(np.float64)
    mu = xs64.mean(axis=1)
    var = xs64.var(axis=1)
    sd = np.sqrt(var + 1e-5)
    y_idx = np.arange(N) % NV
    a_tok = revin_w.astype(np.float64)[y_idx] / sd
    c_tok = revin_b.astype(np.float64)[y_idx] - mu * a_tok
    T0 = _moving_avg(xs64)
    S0 = xs64 - T0

    # ---- host: global expert-grouped pair blocks ----
    nblk_e = np.zeros(E, np.int64)
    lists = []
    for e in range(E):
        l1 = np.nonzero(tk_idx[:, 0] == e)[0]
        l2 = np.nonzero(tk_idx[:, 1] == e)[0]
        lists.append((l1, l2))
        nblk_e[e] = max(1, -(-(len(l1) + len(l2)) // 128))
    total_blocks = int(nblk_e.sum())
    pad_blocks = (-total_blocks) % N_CORES
    nblk_e[E - 1] += pad_blocks
    total_blocks += pad_blocks
    blocks_per_core = total_blocks // N_CORES
    CG = total_blocks * 128

    perm_g = np.zeros(CG, np.int64)
    gsc_g = np.zeros(CG, np.float64)
    r1_g = np.empty(N, np.int64)
    r2_g = np.empty(N, np.int64)
    expert_of_block = np.empty(total_blocks, np.int64)
    off = 0
    boff = 0
    for e in range(E):
        l1, l2 = lists[e]
        seg = np.concatenate([l1, l2])
        perm_g[off:off + len(seg)] = seg
        gsc_g[off:off + len(l1)] = tk_gates[l1, 0]
        gsc_g[off + len(l1):off + len(seg)] = tk_gates[l2, 1]
        r1_g[l1] = off + np.arange(len(l1))
        r2_g[l2] = off + len(l1) + np.arange(len(l2))
        expert_of_block[boff:boff + nblk_e[e]] = e
        off += int(nblk_e[e]) * 128
        boff += int(nblk_e[e])

    ga = gsc_g * a_tok[perm_g]
    # zero the pad columns (anything past each expert's real count)
    real = np.zeros(CG, bool)
    off = 0
    for e in range(E):
        l1, l2 = lists[e]
        real[off:off + len(l1) + len(l2)] = True
        off += int(nblk_e[e]) * 128
    ga[~real] = 0.0

    XSg = (S0[perm_g] * ga[:, None]).T.astype(np.float16)  # (L, CG)
    XTg = (T0[perm_g] * ga[:, None]).T.astype(np.float16)
    XSg = np.ascontiguousarray(XSg).reshape(L // 128, 128, CG)
    XTg = np.ascontiguousarray(XTg).reshape(L // 128, 128, CG)

    U = Wt.sum(axis=2).astype(np.float64)
    Gc = gates.astype(np.float64) * c_tok[:, None]
    cterm = Gc @ U

    WST = np.ascontiguousarray(Ws.transpose(0, 2, 1)).astype(np.float16)
    WTT = np.ascontiguousarray(Wt.transpose(0, 2, 1)).astype(np.float16)

    # ---- per-core programs + inputs ----
    C_core = blocks_per_core * 128
    ncs = []
    in_maps = []
    for c in range(N_CORES):
        b0 = c * blocks_per_core
        ebs = expert_of_block[b0:b0 + blocks_per_core]
        runs = []
        for e in ebs:
            if runs and runs[-1][0] == e:
                runs[-1][1] += 1
            else:
                runs.append([int(e), 1])
        Wc = np.empty((len(runs), 2, L // 128, 128, P), np.float16)
        for r, (e, _nb) in enumerate(runs):
            Wc[r, 0] = WST[e].reshape(L // 128, 128, P)
            Wc[r, 1] = WTT[e].reshape(L // 128, 128, P)
        nc = _build_core_program(tuple(nb for _e, nb in runs), C_core, core=c)
        ncs.append(nc)
        sl = slice(b0 * 128, (b0 + blocks_per_core) * 128)
        in_maps.append({
            "XS": np.ascontiguousarray(XSg[:, :, sl]),
            "XT": np.ascontiguousarray(XTg[:, :, sl]),
            "W": Wc,
        })

    # ---- run all 8 cores concurrently (per-device pinned threads) ----
    bass2jax.install_neuronx_cc_hook()
    devices = jax.devices()
    results = [None] * N_CORES
    errs = [None] * N_CORES

    def worker(c):
        try:
            with jax.default_device(devices[c]):
                res = bass2jax.run_bass_via_pjrt(ncs[c], [in_maps[c]], n_cores=1)
            results[c] = res[0]["O"]
        except Exception as ex:  # noqa: BLE001
            errs[c] = ex

    threads = [threading.Thread(target=worker, args=(c,)) for c in range(N_CORES)]
    for t in threads:
        t.start()
    for t in threads:
        t.join()
    for c in range(N_CORES):
        if errs[c] is not None:
            raise errs[c]

    # ---- host: combine ----
    O_g = np.concatenate(results, axis=0).astype(np.float64)  # (CG, P)
    y = (O_g[r1_g] + O_g[r2_g] + cterm).astype(np.float32)

    return y[..., None].astype(np.float32), loss


# revision 8
# speedup vs baseline: 30338.7107x; 1.0590x over previous
"""MoE Trainium2 kernel v3: expert-balanced pair sharding, per-core programs.

All (token, expert) pairs are grouped by expert into 128-row blocks globally,
then the block list is cut into 8 equal contiguous spans (one per core).
Each core runs its OWN compiled program (runs of consecutive same-expert
blocks -> one weight load per run), dispatched concurrently on its pinned
device via threads. Host combines y[n] = O[r1]+O[r2]+cterm as before.
"""

import sys

sys.path.insert(0, "/opt/trn_rl_repo")

import threading

import numpy as np

BATCH = 2048
NV = 7
N = BATCH * NV
L = 512
P = 512
E = 8
K = 2
HID = 256
MA = 25
N_CORES = 8

_BASS_CACHE = {}


def _host_gating(xs, gw1, gw2, loss_coef):
    """Replicate the reference gating bit-for-bit (same jnp ops)."""
    import jax
    import jax.numpy as jnp

    xs = jnp.asarray(xs)
    clean_logits = jnp.maximum(xs @ jnp.asarray(gw1).T, 0.0) @ jnp.asarray(gw2).T
    probs = jax.nn.softmax(clean_logits, axis=1)
    top_vals, top_idx = jax.lax.top_k(probs, K + 1)
    tk_vals = top_vals[:, :K]
    tk_idx = top_idx[:, :K]
    tk_gates = tk_vals / (jnp.sum(tk_vals, axis=1, keepdims=True) + 1e-6)
    gates = jnp.zeros_like(probs).at[jnp.arange(N)[:, None], tk_idx].set(tk_gates)
    importance = jnp.sum(gates, axis=0)
    load = jnp.sum((gates > 0).astype(jnp.float32), axis=0)

    def _cv(v):
        return jnp.var(v, ddof=1) / (jnp.mean(v) ** 2 + 1e-10)

    loss = (_cv(importance) + _cv(load)) * loss_coef
    return (
        np.asarray(tk_idx),
        np.asarray(tk_gates),
        np.asarray(gates),
        np.asarray(loss),
    )


def _moving_avg(xs64):
    pad = (MA - 1) // 2
    xp = np.concatenate(
        [np.repeat(xs64[:, :1], pad, 1), xs64, np.repeat(xs64[:, -1:], pad, 1)], axis=1
    )
    cs = np.cumsum(np.pad(xp, ((0, 0), (1, 0))), axis=1)
    return (cs[:, MA:] - cs[:, :-MA]) / MA


def _build_core_program(runs, C_core, chunk_blocks=2, evict_batch=2, core=0):
    """One core's program. runs = tuple of (nblk,) per weight-run."""
    key = (tuple(runs), C_core, chunk_blocks, evict_batch, core)
    if key in _BASS_CACHE:
        return _BASS_CACHE[key]

    import concourse.mybir as mybir
    import concourse.tile as tile
    from concourse import bacc

    F32 = mybir.dt.float32
    F16 = mybir.dt.float16
    NLT = L // 128
    n_runs = len(runs)

    nc = bacc.Bacc(None, target_bir_lowering=False, debug=False)
    XS = nc.dram_tensor("XS", [NLT, 128, C_core], F16, kind="ExternalInput")
    XT = nc.dram_tensor("XT", [NLT, 128, C_core], F16, kind="ExternalInput")
    W = nc.dram_tensor("W", [n_runs, 2, NLT, 128, P], F16, kind="ExternalInput")
    O = nc.dram_tensor("O", [C_core, P], F16, kind="ExternalOutput")

    with tile.TileContext(nc) as tc:
        with (
            tc.tile_pool(name="acts", bufs=3) as acts,
            tc.tile_pool(name="wpool", bufs=2) as wpool,
            tc.tile_pool(name="opool", bufs=3) as opool,
            tc.tile_pool(name="psum", bufs=8, space="PSUM") as psum,
        ):
            wsc = acts.tile([128, 512], F16, tag="warm")
            nc.vector.memset(wsc[:], 0.0)
            wps = psum.tile([128, P], F32, tag="acc")
            for _wi in range(11):
                nc.tensor.matmul(wps[:], wsc[:, :128], wsc[:], start=(_wi == 0), stop=(_wi == 10))
            blk0 = 0
            for r, nblk in enumerate(runs):
                w = wpool.tile([128, 2, NLT, P], F16, tag="w")
                nc.sync.dma_start(out=w[:], in_=W[r].rearrange("s a p c -> p s a c"))
                for cb in range(0, nblk, chunk_blocks):
                    ncb = min(chunk_blocks, nblk - cb)
                    cols = ncb * 128
                    col0 = (blk0 + cb) * 128
                    xs_t = acts.tile([128, NLT, chunk_blocks * 128], F16, tag="xs")
                    xt_t = acts.tile([128, NLT, chunk_blocks * 128], F16, tag="xt")
                    nc.sync.dma_start(
                        out=xs_t[:, :, :cols],
                        in_=XS[:, :, col0:col0 + cols].rearrange("a p c -> p a c"),
                    )
                    nc.sync.dma_start(
                        out=xt_t[:, :, :cols],
                        in_=XT[:, :, col0:col0 + cols].rearrange("a p c -> p a c"),
                    )
                    for bg in range(0, ncb, evict_batch):
                        bcnt = min(evict_batch, ncb - bg)
                        ot = opool.tile([128, evict_batch, P], F16, tag="ot")
                        for bi in range(bcnt):
                            c0 = (bg + bi) * 128
                            acc = psum.tile([128, P], F32)
                            for lt in range(NLT):
                                nc.tensor.matmul(
                                    acc[:], xs_t[:, lt, c0:c0 + 128], w[:, 0, lt, :],
                                    start=(lt == 0), stop=False,
                                )
                            for lt in range(NLT):
                                nc.tensor.matmul(
                                    acc[:], xt_t[:, lt, c0:c0 + 128], w[:, 1, lt, :],
                                    start=False, stop=(lt == NLT - 1),
                                )
                            nc.any.tensor_copy(ot[:, bi, :], acc[:])
                        oc0 = col0 + bg * 128
                        nc.gpsimd.dma_start(
                            out=O[oc0:oc0 + bcnt * 128, :].rearrange(
                                "(b p) c -> p b c", p=128
                            ),
                            in_=ot[:, :bcnt, :],
                        )
                blk0 += nblk

    nc.compile()
    _BASS_CACHE[key] = nc
    return nc


def kernel(x, gw1, gw2, Ws, Wt, revin_w, revin_b, loss_coef):
    import jax
    from concourse import bass2jax

    x = np.asarray(x)
    gw1 = np.asarray(gw1)
    gw2 = np.asarray(gw2)
    Ws = np.asarray(Ws)
    Wt = np.asarray(Wt)
    revin_w = np.asarray(revin_w)
    revin_b = np.asarray(revin_b)

    xs = x[..., 0]  # (N, L) f32

    # ---- host: gating + loss (bit-identical to reference) ----
    tk_idx, tk_gates, gates, loss = _host_gating(xs, gw1, gw2, loss_coef)

    # ---- host: RevIN affine + decomposition (linear prep, f64) ----
    xs64 = xs.astype(np.float64)
    mu = xs64.mean(axis=1)
    var = xs64.var(axis=1)
    sd = np.sqrt(var + 1e-5)
    y_idx = np.arange(N) % NV
    a_tok = revin_w.astype(np.float64)[y_idx] / sd
    c_tok = revin_b.astype(np.float64)[y_idx] - mu * a_tok
    T0 = _moving_avg(xs64)
    S0 = xs64 - T0

    # ---- host: global expert-grouped pair blocks ----
    nblk_e = np.zeros(E, np.int64)
    lists = []
    for e in range(E):
        l1 = np.nonzero(tk_idx[:, 0] == e)[0]
        l2 = np.nonzero(tk_idx[:, 1] == e)[0]
        lists.append((l1, l2))
        nblk_e[e] = max(1, -(-(len(l1) + len(l2)) // 128))
    total_blocks = int(nblk_e.sum())
    pad_blocks = (-total_blocks) % N_CORES
    nblk_e[E - 1] += pad_blocks
    total_blocks += pad_blocks
    blocks_per_core = total_blocks // N_CORES
    CG = total_blocks * 128

    perm_g = np.zeros(CG, np.int64)
    gsc_g = np.zeros(CG, np.float64)
    r1_g = np.empty(N, np.int64)
    r2_g = np.empty(N, np.int64)
    expert_of_block = np.empty(total_blocks, np.int64)
    off = 0
    boff = 0
    for e in range(E):
        l1, l2 = lists[e]
        seg = np.concatenate([l1, l2])
        perm_g[off:off + len(seg)] = seg
        gsc_g[off:off + len(l1)] = tk_gates[l1, 0]
        gsc_g[off + len(l1):off + len(seg)] = tk_gates[l2, 1]
        r1_g[l1] = off + np.arange(len(l1))
        r2_g[l2] = off + len(l1) + np.arange(len(l2))
        expert_of_block[boff:boff + nblk_e[e]] = e
        off += int(nblk_e[e]) * 128
        boff += int(nblk_e[e])

    ga = gsc_g * a_tok[perm_g]
    # zero the pad columns (anything past each expert's real count)
    real = np.zeros(CG, bool)
    off = 0
    for e in range(E):
        l1, l2 = lists[e]
        real[off:off + len(l1) + len(l2)] = True
        off += int(nblk_e[e]) * 128
    ga[~real] = 0.0

    XSg = (S0[perm_g] * ga[:, None]).T.astype(np.float16)  # (L, CG)
    XTg = (T0[perm_g] * ga[:, None]).T.astype(np.float16)
    XSg = np.ascontiguousarray(XSg).reshape(L // 128, 128, CG)
    XTg = np.ascontiguousarray(XTg).reshape(L // 128, 128, CG)

    U = Wt.sum(axis=2).astype(np.float64)
    Gc = gates.astype(np.float64) * c_tok[:, None]
    cterm = Gc @ U

    WST = np.ascontiguousarray(Ws.transpose(0, 2, 1)).astype(np.float16)
    WTT = np.ascontiguousarray(Wt.transpose(0, 2, 1)).astype(np.float16)

    # ---- per-core programs + inputs ----
    C_core = blocks_per_core * 128
    ncs = []
    in_maps = []
    for c in range(N_CORES):
        b0 = c * blocks_per_core
        ebs = expert_of_block[b0:b0 + blocks_per_core]
        runs = []
        for e in ebs:
            if runs and runs[-1][0] == e:
                runs[-1][1] += 1
            else:
                runs.append([int(e), 1])
        Wc = np.empty((len(runs), 2, L // 128, 128, P), np.float16)
        for r, (e, _nb) in enumerate(runs):
            Wc[r, 0] = WST[e].reshape(L // 128, 128, P)
            Wc[r, 1] = WTT[e].reshape(L // 128, 128, P)
        nc = _build_core_program(tuple(nb for _e, nb in runs), C_core, core=c)
        ncs.append(nc)
        sl = slice(b0 * 128, (b0 + blocks_per_core) * 128)
        in_maps.append({
            "XS": np.ascontiguousarray(XSg[:, :, sl]),
            "XT": np.ascontiguousarray(XTg[:, :, sl]),
            "W": Wc,
        })

    # ---- run all 8 cores concurrently (per-device pinned threads) ----
    bass2jax.install_neuronx_cc_hook()
    devices = jax.devices()
    results = [None] * N_CORES
    errs = [None] * N_CORES

    def worker(c):
        try:
            with jax.default_device(devices[c]):
                res = bass2jax.run_bass_via_pjrt(ncs[c], [in_maps[c]], n_cores=1)
            results[c] = res[0]["O"]
        except Exception as ex:  # noqa: BLE001
            errs[c] = ex

    threads = [threading.Thread(target=worker, args=(c,)) for c in range(N_CORES)]
    for t in threads:
        t.start()
    for t in threads:
        t.join()
    for c in range(N_CORES):
        if errs[c] is not None:
            raise errs[c]

    # ---- host: combine ----
    O_g = np.concatenate(results, axis=0).astype(np.float64)  # (CG, P)
    y = (O_g[r1_g] + O_g[r2_g] + cterm).astype(np.float32)

    return y[..., None].astype(np.float32), loss
